# revision 3
# baseline (speedup 1.0000x reference)
"""Trainium2 Bass kernel for nn_DiffusionLayer (N=8192, D=128), 8-core SPMD.

Computation:
    t = relu(Z @ W1 + b1) @ W2 + b2      # [N, D]
    S = softmax(t @ t.T, axis=1)         # [N, N]
    out = Z + TAU * (S @ Z - Z)

Sharding: rows of the output are split across 8 NeuronCores. Every core
receives the full Z (needed as keys/values) plus its own 1024-row block,
computes t for all N locally (cheap, avoids collectives), then the
flash-attention-style softmax(t_blk @ t.T) @ Z for its block.

Per-core pipeline:
  - DMA Z in natural row-tile layout; PE-transpose to Z^T.
  - 2-layer MLP in fp32r (full-speed matmul, ~1.6e-4 matmul accuracy),
    bias+relu fused into the DVE PSUM-drain op -> t^T [d, N] in SBUF.
  - sim^T tiles [j-tile 128, i-chunk 256] via fp32r matmuls, grouped 6
    j-tiles per 3-bank PSUM group.
  - exp on ScalarE with a host-computed constant shift -C as the free
    activation bias (softmax is shift invariant; C keeps exp in fp32
    range). Output E in bf16.
  - PV: E-slice^T @ [Z | 1] in bf16, accumulated over all 64 j-tiles in
    PSUM; the appended ones column makes the softmax denominator fall out
    as output column 128. Interleaved with the next chunk's sim matmuls.
  - normalize + residual on DVE, DMA the block out.
"""

import sys

sys.path.insert(0, "/opt/trn_rl_repo")

import numpy as np
import orjson
from contextlib import ExitStack

import concourse.bass as bass
import concourse.tile as tile
from concourse import mybir
from concourse.bass_utils import run_bass_kernel_spmd

F32 = mybir.dt.float32
F32R = mybir.dt.float32r
BF16 = mybir.dt.bfloat16

N, D = 8192, 128
NCORES = 8
BLK = N // NCORES  # 1024 rows per core
NT = N // 128  # 64 row tiles of full Z
NBT = BLK // 128  # 8 row tiles of the block
TAU = 0.1

CH = 256  # i-chunk width for sim/exp/PV
NCH = BLK // CH  # 4 chunks per core
GJ = 6  # j-tiles per sim PSUM group (3 banks)

# ---------------------------------------------------------------------------
# BIR post-pass: the walrus build in this image encodes at most one sync wait
# per instruction; Tile emits several on the kernel-tail drain. Split excess
# waits onto preceding same-engine Drain carriers (engines execute their
# stream in order, so this preserves semantics).
_MAX_WAITS = 1


def _split_multiwaits(m: dict) -> bool:
    changed = False
    counter = [0]

    def fresh_name():
        counter[0] += 1
        return f"I-waitsplit-{counter[0]}"

    for fn in m.get("functions", []):
        for bb in fn.get("blocks", []):
            out = []
            for inst in bb.get("instructions", []):
                si = inst.get("sync_info") or {}
                waits = si.get("on_wait") or []
                if len(waits) > _MAX_WAITS:
                    changed = True
                    head, tail = waits[:-_MAX_WAITS], waits[-_MAX_WAITS:]
                    for i in range(0, len(head), _MAX_WAITS):
                        out.append(
                            {
                                "debug": inst.get("debug", 0),
                                "engine": inst["engine"],
                                "ins": [],
                                "is_reset_sema": False,
                                "name": fresh_name(),
                                "opcode": "Drain",
                                "outs": [],
                                "sync_info": {
                                    "on_update": [],
                                    "on_wait": head[i : i + _MAX_WAITS],
                                },
                            }
                        )
                    si["on_wait"] = tail
                out.append(inst)
            bb["instructions"] = out
    return changed


def _patch_nc(nc):
    orig = nc.to_json_bytes

    def to_json_bytes_fixed():
        m = orjson.loads(orig())
        if _split_multiwaits(m):
            return orjson.dumps(m)
        return orig()

    nc.to_json_bytes = to_json_bytes_fixed
    return nc


# ---------------------------------------------------------------------------


def _build_nc(c_shift: float):
    from concourse.masks import make_identity

    nc = bass.Bass("TRN2", debug=False, num_devices=NCORES)

    Zd = nc.dram_tensor("Z", [N, D], F32, kind="ExternalInput").ap()
    Zbd = nc.dram_tensor("Zb", [BLK, D], F32, kind="ExternalInput").ap()
    W1d = nc.dram_tensor("W1", [D, D], F32, kind="ExternalInput").ap()
    W2d = nc.dram_tensor("W2", [D, D], F32, kind="ExternalInput").ap()
    b1d = nc.dram_tensor("b1", [D, 1], F32, kind="ExternalInput").ap()
    b2d = nc.dram_tensor("b2", [D, 1], F32, kind="ExternalInput").ap()
    Od = nc.dram_tensor("O", [BLK, D], F32, kind="ExternalOutput").ap()

    Zr = Zd.rearrange("(t p) d -> p t d", p=128)  # [128, 64, 128]
    Zbr = Zbd.rearrange("(t p) d -> p t d", p=128)  # [128, 8, 128]
    Or = Od.rearrange("(t p) d -> p t d", p=128)

    with tile.TileContext(nc) as tc, ExitStack() as ctx:
        const = ctx.enter_context(tc.tile_pool(name="const", bufs=1))
        sb = ctx.enter_context(tc.tile_pool(name="sb", bufs=1))
        big = ctx.enter_context(tc.tile_pool(name="big", bufs=3))
        simps = ctx.enter_context(tc.tile_pool(name="simps", bufs=2, space="PSUM"))
        pvps = ctx.enter_context(tc.tile_pool(name="pvps", bufs=2, space="PSUM"))

        # ---- constants / small tiles
        ident = const.tile([128, 128], F32)
        make_identity(nc, ident[:])

        dummy = const.tile([128, 1], F32)
        nc.vector.memset(dummy[:], 0.0)
        dummy2 = const.tile([128, 1], F32)
        # preload the exp table set early so the first real exp doesn't stall
        nc.scalar.activation(dummy2[:], dummy[:], mybir.ActivationFunctionType.Exp)

        cbias = const.tile([128, 1], F32)  # per-partition exp bias = -C
        nc.vector.memset(cbias[:], -c_shift)

        w1s = const.tile([128, 128], F32)
        w2s = const.tile([128, 128], F32)
        b1s = const.tile([128, 1], F32)
        b2s = const.tile([128, 1], F32)
        nc.sync.dma_start(w1s[:], W1d)
        nc.sync.dma_start(w2s[:], W2d)
        nc.sync.dma_start(b1s[:], b1d)
        nc.sync.dma_start(b2s[:], b2d)
        w1r = const.tile([128, 128], F32R)
        w2r = const.tile([128, 128], F32R)
        nc.vector.tensor_copy(w1r[:], w1s[:])
        nc.vector.tensor_copy(w2r[:], w2s[:])

        # ---- persistent SBUF tensors
        t_sb = sb.tile([128, N], F32R)  # t^T [d, N]
        zaug = sb.tile([128, NT, D + 1], BF16)  # [Z | 1] row tiles, bf16
        zbn = sb.tile([128, NBT, 128], F32)  # Z block natural (residual)
        zbt = sb.tile([128, BLK], F32R)  # Zblk^T
        hb_sb = sb.tile([128, BLK], F32R)
        tb_sb = sb.tile([128, BLK], F32R)  # t_blk^T [d, BLK]
        u_sb = sb.tile([128, NBT, D + 1], F32)  # unnormalized PV + denom
        o_sb = sb.tile([128, NBT, 128], F32)
        rec = sb.tile([128, NBT, 1], F32)

        # ---- big rotating tiles (32KB/partition each, bufs=3)
        zn = big.tile([128, NT, 128], F32, tag="big")  # Z natural
        zt = big.tile([128, N], F32R, tag="big")  # Z^T
        h_sb = big.tile([128, N], F32R, tag="big")  # hidden^T

        # ---- load Z natural, 16 DMAs of 4 row-tiles each
        for g in range(16):
            nc.sync.dma_start(zn[:, 4 * g : 4 * g + 4, :], Zr[:, 4 * g : 4 * g + 4, :])
        nc.sync.dma_start(zbn[:, 0:4, :], Zbr[:, 0:4, :])
        nc.sync.dma_start(zbn[:, 4:8, :], Zbr[:, 4:8, :])

        # ---- transpose Z -> Z^T (PE transpose, groups of 4 tiles per bank)
        for g in range(16):
            ps = pvps.tile([128, 4, 128], F32, tag="ps")
            for k in range(4):
                nc.tensor.transpose(ps[:, k, :], zn[:, 4 * g + k, :], ident[:])
            nc.vector.tensor_copy(zt[:, 512 * g : 512 * (g + 1)], ps[:])

        # ---- Zaug = [Z | 1] in bf16 (PV moving operand)
        for q in range(4):
            nc.vector.tensor_copy(
                zaug[:, 16 * q : 16 * (q + 1), 0:D], zn[:, 16 * q : 16 * (q + 1), :]
            )
        nc.vector.memset(zaug[:, :, D : D + 1], 1.0)

        # ---- MLP on full Z: t^T = W2^T @ relu(W1^T @ Z^T + b1) + b2
        for ch in range(16):
            cs = slice(512 * ch, 512 * (ch + 1))
            p1 = pvps.tile([128, 512], F32, tag="ps")
            nc.tensor.matmul(p1[:], w1r[:], zt[:, cs], start=True, stop=True)
            # fused bias + relu on the PSUM drain
            nc.vector.tensor_scalar(
                h_sb[:, cs], p1[:], b1s[:], 0.0, mybir.AluOpType.add,
                mybir.AluOpType.max,
            )
            p2 = pvps.tile([128, 512], F32, tag="ps")
            nc.tensor.matmul(p2[:], w2r[:], h_sb[:, cs], start=True, stop=True)
            nc.vector.tensor_scalar_add(t_sb[:, cs], p2[:], b2s[:])

        # ---- same for the block rows -> tb (sim moving operand)
        for g in range(2):
            ps = pvps.tile([128, 4, 128], F32, tag="ps")
            for k in range(4):
                nc.tensor.transpose(ps[:, k, :], zbn[:, 4 * g + k, :], ident[:])
            nc.vector.tensor_copy(zbt[:, 512 * g : 512 * (g + 1)], ps[:])
        for ch in range(2):
            cs = slice(512 * ch, 512 * (ch + 1))
            p1 = pvps.tile([128, 512], F32, tag="ps")
            nc.tensor.matmul(p1[:], w1r[:], zbt[:, cs], start=True, stop=True)
            nc.vector.tensor_scalar(
                hb_sb[:, cs], p1[:], b1s[:], 0.0, mybir.AluOpType.add,
                mybir.AluOpType.max,
            )
            p2 = pvps.tile([128, 512], F32, tag="ps")
            nc.tensor.matmul(p2[:], w2r[:], hb_sb[:, cs], start=True, stop=True)
            nc.vector.tensor_scalar_add(tb_sb[:, cs], p2[:], b2s[:])

        # ---- sim + exp + PV, chunked over i
        groups = []
        off = 0
        while off < NT:
            groups.append((off, min(GJ, NT - off)))
            off += GJ

        e_tiles = [None] * NCH

        def emit_sim_exp(c):
            ic = slice(CH * c, CH * (c + 1))
            e_sb = big.tile([128, NT, CH], BF16, tag="big", name=f"e_{c}")
            e_tiles[c] = e_sb
            for go, gn in groups:
                ps = simps.tile([128, GJ, CH], F32, tag="simps")
                for k in range(gn):
                    jt = go + k
                    nc.tensor.matmul(
                        ps[:, k, :],
                        t_sb[:, 128 * jt : 128 * (jt + 1)],
                        tb_sb[:, ic],
                        start=True,
                        stop=True,
                    )
                nc.scalar.activation(
                    e_sb[:, go : go + gn, :],
                    ps[:, 0:gn, :],
                    mybir.ActivationFunctionType.Exp,
                    bias=cbias[:],
                )

        def emit_pv(c):
            e_sb = e_tiles[c]
            for s in (2 * c, 2 * c + 1):
                si = (s % 2) * 128
                pv = pvps.tile([128, D + 1], F32, tag="ps", name=f"pv_{s}")
                for jt in range(NT):
                    nc.tensor.matmul(
                        pv[:],
                        e_sb[:, jt, si : si + 128],
                        zaug[:, jt, :],
                        start=(jt == 0),
                        stop=(jt == NT - 1),
                    )
                nc.vector.tensor_copy(u_sb[:, s, :], pv[:])

        for c in range(NCH):
            emit_sim_exp(c)
            if c > 0:
                emit_pv(c - 1)
        emit_pv(NCH - 1)

        # ---- normalize + residual
        nc.vector.reciprocal(rec[:], u_sb[:, :, D : D + 1])
        nc.vector.tensor_scalar_mul(rec[:], rec[:], TAU)
        for s in range(NBT):
            nc.vector.tensor_scalar_mul(
                u_sb[:, s, 0:D], u_sb[:, s, 0:D], rec[:, s, :]
            )
            nc.vector.scalar_tensor_tensor(
                o_sb[:, s, :],
                zbn[:, s, :],
                1.0 - TAU,
                u_sb[:, s, 0:D],
                mybir.AluOpType.mult,
                mybir.AluOpType.add,
            )
        nc.sync.dma_start(Or[:], o_sb[:])

    return _patch_nc(nc)


# ---------------------------------------------------------------------------

_CACHE = {}


def _get_nc(c_shift: float):
    key = round(float(c_shift), 3)
    if key not in _CACHE:
        _CACHE[key] = _build_nc(key)
    return _CACHE[key]


def kernel(Z, W1, b1, W2, b2):
    Z = np.ascontiguousarray(np.asarray(Z, dtype=np.float32))
    W1 = np.ascontiguousarray(np.asarray(W1, dtype=np.float32))
    W2 = np.ascontiguousarray(np.asarray(W2, dtype=np.float32))
    b1 = np.asarray(b1, dtype=np.float32).reshape(D, 1)
    b2 = np.asarray(b2, dtype=np.float32).reshape(D, 1)

    # Host-side t (cheap) to pick the constant softmax shift C: keeps
    # exp(sim - C) inside fp32 range. sim <= max||t||^2 (Cauchy-Schwarz),
    # row maxima >= diag = ||t_i||^2.
    t = np.maximum(Z @ W1 + b1.T, 0.0) @ W2 + b2.T
    d2 = np.einsum("nd,nd->n", t, t)
    c_shift = float(min(max(d2.max() - 85.0, 0.0), d2.min() + 80.0))

    nc = _get_nc(c_shift)
    in_maps = []
    for c in range(NCORES):
        in_maps.append(
            {
                "Z": Z,
                "Zb": Z[c * BLK : (c + 1) * BLK],
                "W1": W1,
                "W2": W2,
                "b1": b1,
                "b2": b2,
            }
        )
    res = run_bass_kernel_spmd(nc, in_maps, list(range(NCORES)))
    return np.concatenate([res.results[c]["O"] for c in range(NCORES)], axis=0)


# revision 4
# speedup vs baseline: 1.1067x; 1.1067x over previous
"""Trainium2 Bass kernel for nn_DiffusionLayer (N=8192, D=128), 8-core SPMD.

Computation:
    t = relu(Z @ W1 + b1) @ W2 + b2      # [N, D]
    S = softmax(t @ t.T, axis=1)         # [N, N]
    out = Z + TAU * (S @ Z - Z)

Sharding: rows of the output are split across 8 NeuronCores. Every core
receives the full Z (needed as keys/values) plus its own 1024-row block,
computes t for all N locally (cheap, avoids collectives), then the
flash-attention-style softmax(t_blk @ t.T) @ Z for its block.

Per-core pipeline:
  - DMA Z in natural row-tile layout; PE-transpose to Z^T.
  - 2-layer MLP in fp32r (full-speed matmul, ~1.6e-4 matmul accuracy),
    bias+relu fused into the DVE PSUM-drain op -> t^T [d, N] in SBUF.
  - sim^T tiles [j-tile 128, i-chunk 256] via fp32r matmuls, grouped 6
    j-tiles per 3-bank PSUM group.
  - exp on ScalarE with a host-computed constant shift -C as the free
    activation bias (softmax is shift invariant; C keeps exp in fp32
    range). Output E in bf16.
  - PV: E-slice^T @ [Z | 1] in bf16, accumulated over all 64 j-tiles in
    PSUM; the appended ones column makes the softmax denominator fall out
    as output column 128. Interleaved with the next chunk's sim matmuls.
  - normalize + residual on DVE, DMA the block out.
"""

import sys

sys.path.insert(0, "/opt/trn_rl_repo")

import numpy as np
import orjson
from contextlib import ExitStack

import concourse.bass as bass
import concourse.tile as tile
from concourse import mybir
from concourse.bass_utils import run_bass_kernel_spmd

F32 = mybir.dt.float32
F32R = mybir.dt.float32r
BF16 = mybir.dt.bfloat16

N, D = 8192, 128
NCORES = 8
BLK = N // NCORES  # 1024 rows per core
NT = N // 128  # 64 row tiles of full Z
NBT = BLK // 128  # 8 row tiles of the block
TAU = 0.1

CH = 256  # i-chunk width for sim/exp/PV
NCH = BLK // CH  # 4 chunks per core
GJ = 6  # j-tiles per sim PSUM group (3 banks)

# ---------------------------------------------------------------------------
# BIR post-pass: the walrus build in this image encodes at most one sync wait
# per instruction; Tile emits several on the kernel-tail drain. Split excess
# waits onto preceding same-engine Drain carriers (engines execute their
# stream in order, so this preserves semantics).
_MAX_WAITS = 1


def _split_multiwaits(m: dict) -> bool:
    changed = False
    counter = [0]

    def fresh_name():
        counter[0] += 1
        return f"I-waitsplit-{counter[0]}"

    for fn in m.get("functions", []):
        for bb in fn.get("blocks", []):
            out = []
            for inst in bb.get("instructions", []):
                si = inst.get("sync_info") or {}
                waits = si.get("on_wait") or []
                if len(waits) > _MAX_WAITS:
                    changed = True
                    head, tail = waits[:-_MAX_WAITS], waits[-_MAX_WAITS:]
                    for i in range(0, len(head), _MAX_WAITS):
                        # NoOp (not Drain): stalls dispatch on the sem wait
                        # without flushing the engine pipeline.
                        out.append(
                            {
                                "debug": inst.get("debug", 0),
                                "engine": inst["engine"],
                                "ins": [],
                                "name": fresh_name(),
                                "opcode": "NoOp",
                                "outs": [],
                                "sync_info": {
                                    "on_update": [],
                                    "on_wait": head[i : i + _MAX_WAITS],
                                },
                            }
                        )
                    si["on_wait"] = tail
                out.append(inst)
            bb["instructions"] = out
    return changed


def _patch_nc(nc):
    orig = nc.to_json_bytes

    def to_json_bytes_fixed():
        m = orjson.loads(orig())
        if _split_multiwaits(m):
            return orjson.dumps(m)
        return orig()

    nc.to_json_bytes = to_json_bytes_fixed
    return nc


# ---------------------------------------------------------------------------


def _build_nc(c_shift: float):
    from concourse.masks import make_identity

    nc = bass.Bass("TRN2", debug=False, num_devices=NCORES)

    Zd = nc.dram_tensor("Z", [N, D], F32, kind="ExternalInput").ap()
    Zbd = nc.dram_tensor("Zb", [BLK, D], F32, kind="ExternalInput").ap()
    W1d = nc.dram_tensor("W1", [D, D], F32, kind="ExternalInput").ap()
    W2d = nc.dram_tensor("W2", [D, D], F32, kind="ExternalInput").ap()
    b1d = nc.dram_tensor("b1", [D, 1], F32, kind="ExternalInput").ap()
    b2d = nc.dram_tensor("b2", [D, 1], F32, kind="ExternalInput").ap()
    Od = nc.dram_tensor("O", [BLK, D], F32, kind="ExternalOutput").ap()

    Zr = Zd.rearrange("(t p) d -> p t d", p=128)  # [128, 64, 128]
    Zbr = Zbd.rearrange("(t p) d -> p t d", p=128)  # [128, 8, 128]
    Or = Od.rearrange("(t p) d -> p t d", p=128)

    with tile.TileContext(nc) as tc, ExitStack() as ctx:
        const = ctx.enter_context(tc.tile_pool(name="const", bufs=1))
        sb = ctx.enter_context(tc.tile_pool(name="sb", bufs=1))
        big = ctx.enter_context(tc.tile_pool(name="big", bufs=3))
        simps = ctx.enter_context(tc.tile_pool(name="simps", bufs=2, space="PSUM"))
        pvps = ctx.enter_context(tc.tile_pool(name="pvps", bufs=2, space="PSUM"))

        # ---- constants / small tiles
        ident = const.tile([128, 128], F32)
        make_identity(nc, ident[:])

        dummy = const.tile([128, 1], F32)
        nc.vector.memset(dummy[:], 0.0)
        dummy2 = const.tile([128, 1], F32)
        # preload the exp table set early so the first real exp doesn't stall
        nc.scalar.activation(dummy2[:], dummy[:], mybir.ActivationFunctionType.Exp)

        cbias = const.tile([128, 1], F32)  # per-partition exp bias = -C
        nc.vector.memset(cbias[:], -c_shift)

        w1s = const.tile([128, 128], F32)
        w2s = const.tile([128, 128], F32)
        b1s = const.tile([128, 1], F32)
        b2s = const.tile([128, 1], F32)
        nc.sync.dma_start(w1s[:], W1d)
        nc.sync.dma_start(w2s[:], W2d)
        nc.sync.dma_start(b1s[:], b1d)
        nc.sync.dma_start(b2s[:], b2d)
        w1r = const.tile([128, 128], F32R)
        w2r = const.tile([128, 128], F32R)
        nc.vector.tensor_copy(w1r[:], w1s[:])
        nc.vector.tensor_copy(w2r[:], w2s[:])

        # ---- persistent SBUF tensors
        t_sb = sb.tile([128, N], F32R)  # t^T [d, N]
        zaug = sb.tile([128, NT, D + 1], BF16)  # [Z | 1] row tiles, bf16
        zbn = sb.tile([128, NBT, 128], F32)  # Z block natural (residual)
        zbt = sb.tile([128, BLK], F32R)  # Zblk^T
        hb_sb = sb.tile([128, BLK], F32R)
        tb_sb = sb.tile([128, BLK], F32R)  # t_blk^T [d, BLK]
        u_sb = sb.tile([128, NBT, D + 1], F32)  # unnormalized PV + denom
        o_sb = sb.tile([128, NBT, 128], F32)
        rec = sb.tile([128, NBT, 1], F32)

        # ---- big rotating tiles (32KB/partition each, bufs=3)
        zn = big.tile([128, NT, 128], F32, tag="big")  # Z natural
        zt = big.tile([128, N], F32R, tag="big")  # Z^T
        h_sb = big.tile([128, N], F32R, tag="big")  # hidden^T

        # ---- load Z natural, 16 DMAs of 4 row-tiles each
        for g in range(16):
            nc.sync.dma_start(zn[:, 4 * g : 4 * g + 4, :], Zr[:, 4 * g : 4 * g + 4, :])
        nc.sync.dma_start(zbn[:, 0:4, :], Zbr[:, 0:4, :])
        nc.sync.dma_start(zbn[:, 4:8, :], Zbr[:, 4:8, :])

        # ---- transpose Z -> Z^T (PE transpose, groups of 4 tiles per bank)
        for g in range(16):
            ps = pvps.tile([128, 4, 128], F32, tag="ps")
            for k in range(4):
                nc.tensor.transpose(ps[:, k, :], zn[:, 4 * g + k, :], ident[:])
            nc.vector.tensor_copy(zt[:, 512 * g : 512 * (g + 1)], ps[:])

        # ---- Zaug = [Z | 1] in bf16 (PV moving operand)
        for q in range(4):
            nc.vector.tensor_copy(
                zaug[:, 16 * q : 16 * (q + 1), 0:D], zn[:, 16 * q : 16 * (q + 1), :]
            )
        nc.vector.memset(zaug[:, :, D : D + 1], 1.0)

        # ---- MLP on full Z: t^T = W2^T @ relu(W1^T @ Z^T + b1) + b2
        for ch in range(16):
            cs = slice(512 * ch, 512 * (ch + 1))
            p1 = pvps.tile([128, 512], F32, tag="ps")
            nc.tensor.matmul(p1[:], w1r[:], zt[:, cs], start=True, stop=True)
            # fused bias + relu on the PSUM drain
            nc.vector.tensor_scalar(
                h_sb[:, cs], p1[:], b1s[:], 0.0, mybir.AluOpType.add,
                mybir.AluOpType.max,
            )
            p2 = pvps.tile([128, 512], F32, tag="ps")
            nc.tensor.matmul(p2[:], w2r[:], h_sb[:, cs], start=True, stop=True)
            nc.vector.tensor_scalar_add(t_sb[:, cs], p2[:], b2s[:])

        # ---- same for the block rows -> tb (sim moving operand)
        for g in range(2):
            ps = pvps.tile([128, 4, 128], F32, tag="ps")
            for k in range(4):
                nc.tensor.transpose(ps[:, k, :], zbn[:, 4 * g + k, :], ident[:])
            nc.vector.tensor_copy(zbt[:, 512 * g : 512 * (g + 1)], ps[:])
        for ch in range(2):
            cs = slice(512 * ch, 512 * (ch + 1))
            p1 = pvps.tile([128, 512], F32, tag="ps")
            nc.tensor.matmul(p1[:], w1r[:], zbt[:, cs], start=True, stop=True)
            nc.vector.tensor_scalar(
                hb_sb[:, cs], p1[:], b1s[:], 0.0, mybir.AluOpType.add,
                mybir.AluOpType.max,
            )
            p2 = pvps.tile([128, 512], F32, tag="ps")
            nc.tensor.matmul(p2[:], w2r[:], hb_sb[:, cs], start=True, stop=True)
            nc.vector.tensor_scalar_add(tb_sb[:, cs], p2[:], b2s[:])

        # ---- sim + exp + PV, chunked over i
        groups = []
        off = 0
        while off < NT:
            groups.append((off, min(GJ, NT - off)))
            off += GJ

        e_tiles = [None] * NCH

        def emit_sim_exp(c):
            ic = slice(CH * c, CH * (c + 1))
            e_sb = big.tile([128, NT, CH], BF16, tag="big", name=f"e_{c}")
            e_tiles[c] = e_sb
            for go, gn in groups:
                ps = simps.tile([128, GJ, CH], F32, tag="simps")
                for k in range(gn):
                    jt = go + k
                    nc.tensor.matmul(
                        ps[:, k, :],
                        t_sb[:, 128 * jt : 128 * (jt + 1)],
                        tb_sb[:, ic],
                        start=True,
                        stop=True,
                    )
                nc.scalar.activation(
                    e_sb[:, go : go + gn, :],
                    ps[:, 0:gn, :],
                    mybir.ActivationFunctionType.Exp,
                    bias=cbias[:],
                )

        def emit_pv(c):
            e_sb = e_tiles[c]
            for s in (2 * c, 2 * c + 1):
                si = (s % 2) * 128
                pv = pvps.tile([128, D + 1], F32, tag="ps", name=f"pv_{s}")
                for jt in range(NT):
                    nc.tensor.matmul(
                        pv[:],
                        e_sb[:, jt, si : si + 128],
                        zaug[:, jt, :],
                        start=(jt == 0),
                        stop=(jt == NT - 1),
                    )
                nc.vector.tensor_copy(u_sb[:, s, :], pv[:])

        for c in range(NCH):
            emit_sim_exp(c)
            if c > 0:
                emit_pv(c - 1)
        emit_pv(NCH - 1)

        # ---- normalize + residual
        nc.vector.reciprocal(rec[:], u_sb[:, :, D : D + 1])
        nc.vector.tensor_scalar_mul(rec[:], rec[:], TAU)
        for s in range(NBT):
            nc.vector.tensor_scalar_mul(
                u_sb[:, s, 0:D], u_sb[:, s, 0:D], rec[:, s, :]
            )
            nc.vector.scalar_tensor_tensor(
                o_sb[:, s, :],
                zbn[:, s, :],
                1.0 - TAU,
                u_sb[:, s, 0:D],
                mybir.AluOpType.mult,
                mybir.AluOpType.add,
            )
        nc.sync.dma_start(Or[:], o_sb[:])

    return _patch_nc(nc)


# ---------------------------------------------------------------------------

_CACHE = {}


def _get_nc(c_shift: float):
    key = round(float(c_shift), 3)
    if key not in _CACHE:
        _CACHE[key] = _build_nc(key)
    return _CACHE[key]


def kernel(Z, W1, b1, W2, b2):
    Z = np.ascontiguousarray(np.asarray(Z, dtype=np.float32))
    W1 = np.ascontiguousarray(np.asarray(W1, dtype=np.float32))
    W2 = np.ascontiguousarray(np.asarray(W2, dtype=np.float32))
    b1 = np.asarray(b1, dtype=np.float32).reshape(D, 1)
    b2 = np.asarray(b2, dtype=np.float32).reshape(D, 1)

    # Host-side t (cheap) to pick the constant softmax shift C: keeps
    # exp(sim - C) inside fp32 range. sim <= max||t||^2 (Cauchy-Schwarz),
    # row maxima >= diag = ||t_i||^2.
    t = np.maximum(Z @ W1 + b1.T, 0.0) @ W2 + b2.T
    d2 = np.einsum("nd,nd->n", t, t)
    c_shift = float(min(max(d2.max() - 85.0, 0.0), d2.min() + 80.0))

    nc = _get_nc(c_shift)
    in_maps = []
    for c in range(NCORES):
        in_maps.append(
            {
                "Z": Z,
                "Zb": Z[c * BLK : (c + 1) * BLK],
                "W1": W1,
                "W2": W2,
                "b1": b1,
                "b2": b2,
            }
        )
    res = run_bass_kernel_spmd(nc, in_maps, list(range(NCORES)))
    return np.concatenate([res.results[c]["O"] for c in range(NCORES)], axis=0)


# revision 6
# speedup vs baseline: 1.3715x; 1.2393x over previous
"""Trainium2 Bass kernel for nn_DiffusionLayer (N=8192, D=128), 8-core SPMD.

Computation:
    t = relu(Z @ W1 + b1) @ W2 + b2      # [N, D]
    S = softmax(t @ t.T, axis=1)         # [N, N]
    out = Z + TAU * (S @ Z - Z)

Sharding: rows of the output are split across 8 NeuronCores. Every core
receives the full Z (needed as keys/values) plus its own 1024-row block,
computes t for all N locally (cheap, avoids collectives), then the
flash-attention-style softmax(t_blk @ t.T) @ Z for its block.

Per-core pipeline:
  - DMA Z in natural row-tile layout; PE-transpose to Z^T.
  - 2-layer MLP in fp32r (full-speed matmul, ~1.6e-4 matmul accuracy),
    bias+relu fused into the DVE PSUM-drain op -> t^T [d, N] in SBUF.
  - sim^T tiles [j-tile 128, i-chunk 256] via fp32r matmuls, grouped 6
    j-tiles per 3-bank PSUM group.
  - exp on ScalarE with a host-computed constant shift -C as the free
    activation bias (softmax is shift invariant; C keeps exp in fp32
    range). Output E in bf16.
  - PV: E-slice^T @ [Z | 1] in bf16, accumulated over all 64 j-tiles in
    PSUM; the appended ones column makes the softmax denominator fall out
    as output column 128. Interleaved with the next chunk's sim matmuls.
  - normalize + residual on DVE, DMA the block out.
"""

import sys

sys.path.insert(0, "/opt/trn_rl_repo")

import numpy as np
import orjson
from contextlib import ExitStack

import concourse.bass as bass
import concourse.tile as tile
from concourse import mybir
from concourse.bass_utils import run_bass_kernel_spmd

F32 = mybir.dt.float32
F32R = mybir.dt.float32r
BF16 = mybir.dt.bfloat16

N, D = 8192, 128
NCORES = 8
BLK = N // NCORES  # 1024 rows per core
NT = N // 128  # 64 row tiles of full Z
NBT = BLK // 128  # 8 row tiles of the block
TAU = 0.1

CH = 256  # i-chunk width for sim/exp/PV
NCH = BLK // CH  # 4 chunks per core
GJ = 6  # j-tiles per sim PSUM group (3 banks)

# ---------------------------------------------------------------------------
# BIR post-pass: the walrus build in this image encodes at most one sync wait
# per instruction; Tile emits several on the kernel-tail drain. Split excess
# waits onto preceding same-engine Drain carriers (engines execute their
# stream in order, so this preserves semantics).
_MAX_WAITS = 1


def _split_multiwaits(m: dict) -> bool:
    changed = False
    counter = [0]

    def fresh_name():
        counter[0] += 1
        return f"I-waitsplit-{counter[0]}"

    for fn in m.get("functions", []):
        for bb in fn.get("blocks", []):
            out = []
            for inst in bb.get("instructions", []):
                si = inst.get("sync_info") or {}
                waits = si.get("on_wait") or []
                if len(waits) > _MAX_WAITS:
                    changed = True
                    head, tail = waits[:-_MAX_WAITS], waits[-_MAX_WAITS:]
                    for i in range(0, len(head), _MAX_WAITS):
                        # NoOp (not Drain): stalls dispatch on the sem wait
                        # without flushing the engine pipeline.
                        out.append(
                            {
                                "debug": inst.get("debug", 0),
                                "engine": inst["engine"],
                                "ins": [],
                                "name": fresh_name(),
                                "opcode": "NoOp",
                                "outs": [],
                                "sync_info": {
                                    "on_update": [],
                                    "on_wait": head[i : i + _MAX_WAITS],
                                },
                            }
                        )
                    si["on_wait"] = tail
                out.append(inst)
            bb["instructions"] = out
    return changed


def _patch_nc(nc):
    orig = nc.to_json_bytes

    def to_json_bytes_fixed():
        m = orjson.loads(orig())
        if _split_multiwaits(m):
            return orjson.dumps(m)
        return orig()

    nc.to_json_bytes = to_json_bytes_fixed
    return nc


# ---------------------------------------------------------------------------


def _build_nc(c_shift: float):
    from concourse.masks import make_identity

    nc = bass.Bass("TRN2", debug=False, num_devices=NCORES)

    Zd = nc.dram_tensor("Z", [N, D], F32, kind="ExternalInput").ap()
    Zbd = nc.dram_tensor("Zb", [BLK, D], F32, kind="ExternalInput").ap()
    W1d = nc.dram_tensor("W1", [D, D], F32, kind="ExternalInput").ap()
    W2d = nc.dram_tensor("W2", [D, D], F32, kind="ExternalInput").ap()
    b1d = nc.dram_tensor("b1", [D, 1], F32, kind="ExternalInput").ap()
    b2d = nc.dram_tensor("b2", [D, 1], F32, kind="ExternalInput").ap()
    Od = nc.dram_tensor("O", [BLK, D], F32, kind="ExternalOutput").ap()

    Zr = Zd.rearrange("(t p) d -> p t d", p=128)  # [128, 64, 128]
    Zbr = Zbd.rearrange("(t p) d -> p t d", p=128)  # [128, 8, 128]
    Or = Od.rearrange("(t p) d -> p t d", p=128)

    with tile.TileContext(nc) as tc, ExitStack() as ctx:
        const = ctx.enter_context(tc.tile_pool(name="const", bufs=1))
        sb = ctx.enter_context(tc.tile_pool(name="sb", bufs=1))
        big = ctx.enter_context(tc.tile_pool(name="big", bufs=3))
        simps = ctx.enter_context(tc.tile_pool(name="simps", bufs=2, space="PSUM"))
        pvps = ctx.enter_context(tc.tile_pool(name="pvps", bufs=2, space="PSUM"))

        # ---- constants / small tiles
        ident = const.tile([128, 128], F32)
        make_identity(nc, ident[:])

        dummy = const.tile([128, 1], F32)
        nc.vector.memset(dummy[:], 0.0)
        dummy2 = const.tile([128, 1], F32)
        # preload the exp table set early so the first real exp doesn't stall
        nc.scalar.activation(dummy2[:], dummy[:], mybir.ActivationFunctionType.Exp)

        cbias = const.tile([128, 1], F32)  # per-partition exp bias = -C
        nc.vector.memset(cbias[:], -c_shift)

        w1s = const.tile([128, 128], F32)
        w2s = const.tile([128, 128], F32)
        b1s = const.tile([128, 1], F32)
        b2s = const.tile([128, 1], F32)
        nc.sync.dma_start(w1s[:], W1d)
        nc.sync.dma_start(w2s[:], W2d)
        nc.sync.dma_start(b1s[:], b1d)
        nc.sync.dma_start(b2s[:], b2d)
        w1r = const.tile([128, 128], F32R)
        w2r = const.tile([128, 128], F32R)
        nc.vector.tensor_copy(w1r[:], w1s[:])
        nc.vector.tensor_copy(w2r[:], w2s[:])

        # ---- persistent SBUF tensors
        t_sb = sb.tile([128, N], F32R)  # t^T [d, N]
        zaug = sb.tile([128, NT, D + 1], BF16)  # [Z | 1] row tiles, bf16
        zbn = sb.tile([128, NBT, 128], F32)  # Z block natural (residual)
        zbt = sb.tile([128, BLK], F32R)  # Zblk^T
        hb_sb = sb.tile([128, BLK], F32R)
        tb_sb = sb.tile([128, BLK], F32R)  # t_blk^T [d, BLK]
        u_sb = sb.tile([128, NBT, D + 1], F32)  # unnormalized PV + denom
        o_sb = sb.tile([128, NBT, 128], F32)
        rec = sb.tile([128, NBT, 1], F32)

        # ---- big rotating tiles (32KB/partition each, bufs=3)
        zn = big.tile([128, NT, 128], F32, tag="big")  # Z natural
        zt = big.tile([128, N], F32R, tag="big")  # Z^T
        h_sb = big.tile([128, N], F32R, tag="big")  # hidden^T

        # ---- block rows first: tb is needed by every sim matmul, so get it
        # done before the full-Z pipeline.
        nc.sync.dma_start(zbn[:, 0:4, :], Zbr[:, 0:4, :])
        nc.sync.dma_start(zbn[:, 4:8, :], Zbr[:, 4:8, :])
        for g in range(2):
            ps = simps.tile([128, 4, 128], F32, tag="simps", name=f"trb{g}")
            for k in range(4):
                nc.tensor.transpose(ps[:, k, :], zbn[:, 4 * g + k, :], ident[:])
            nc.vector.tensor_copy(zbt[:, 512 * g : 512 * (g + 1)], ps[:])
        for ch in range(2):
            cs = slice(512 * ch, 512 * (ch + 1))
            p1 = pvps.tile([128, 512], F32, tag="ps")
            nc.tensor.matmul(p1[:], w1r[:], zbt[:, cs], start=True, stop=True)
            nc.vector.tensor_scalar(
                hb_sb[:, cs], p1[:], b1s[:], 0.0, mybir.AluOpType.add,
                mybir.AluOpType.max,
            )
            p2 = pvps.tile([128, 512], F32, tag="ps")
            nc.tensor.matmul(p2[:], w2r[:], hb_sb[:, cs], start=True, stop=True)
            nc.vector.tensor_scalar_add(tb_sb[:, cs], p2[:], b2s[:])

        # ---- load Z natural, 16 DMAs of 4 row-tiles each
        for g in range(16):
            nc.sync.dma_start(zn[:, 4 * g : 4 * g + 4, :], Zr[:, 4 * g : 4 * g + 4, :])

        # ---- transpose Z -> Z^T (PE transpose; drain copies alternate
        # DVE / ACT since both are otherwise idle here)
        for g in range(16):
            ps = simps.tile([128, 4, 128], F32, tag="simps", name=f"trz{g}")
            for k in range(4):
                nc.tensor.transpose(ps[:, k, :], zn[:, 4 * g + k, :], ident[:])
            dst = zt[:, 512 * g : 512 * (g + 1)]
            if g % 2 == 0:
                nc.vector.tensor_copy(dst, ps[:])
            else:
                nc.scalar.copy(dst, ps[:])

        # ---- Zaug = [Z | 1] in bf16 (PV moving operand)
        for q in range(4):
            nc.vector.tensor_copy(
                zaug[:, 16 * q : 16 * (q + 1), 0:D], zn[:, 16 * q : 16 * (q + 1), :]
            )
        nc.vector.memset(zaug[:, :, D : D + 1], 1.0)

        # ---- MLP on full Z: t^T = W2^T @ relu(W1^T @ Z^T + b1) + b2
        for ch in range(16):
            cs = slice(512 * ch, 512 * (ch + 1))
            p1 = pvps.tile([128, 512], F32, tag="ps")
            nc.tensor.matmul(p1[:], w1r[:], zt[:, cs], start=True, stop=True)
            # fused bias + relu on the PSUM drain
            nc.vector.tensor_scalar(
                h_sb[:, cs], p1[:], b1s[:], 0.0, mybir.AluOpType.add,
                mybir.AluOpType.max,
            )
            p2 = pvps.tile([128, 512], F32, tag="ps")
            nc.tensor.matmul(p2[:], w2r[:], h_sb[:, cs], start=True, stop=True)
            nc.vector.tensor_scalar_add(t_sb[:, cs], p2[:], b2s[:])

        # ---- sim + exp + PV, chunked over i
        groups = []
        off = 0
        while off < NT:
            groups.append((off, min(GJ, NT - off)))
            off += GJ

        e_tiles = [None] * NCH

        def emit_sim_exp(c):
            ic = slice(CH * c, CH * (c + 1))
            e_sb = big.tile([128, NT, CH], BF16, tag="big", name=f"e_{c}")
            e_tiles[c] = e_sb
            for go, gn in groups:
                ps = simps.tile([128, GJ, CH], F32, tag="simps")
                for k in range(gn):
                    jt = go + k
                    nc.tensor.matmul(
                        ps[:, k, :],
                        t_sb[:, 128 * jt : 128 * (jt + 1)],
                        tb_sb[:, ic],
                        start=True,
                        stop=True,
                    )
                nc.scalar.activation(
                    e_sb[:, go : go + gn, :],
                    ps[:, 0:gn, :],
                    mybir.ActivationFunctionType.Exp,
                    bias=cbias[:],
                )

        def emit_pv(c):
            e_sb = e_tiles[c]
            for s in (2 * c, 2 * c + 1):
                si = (s % 2) * 128
                pv = pvps.tile([128, D + 1], F32, tag="ps", name=f"pv_{s}")
                for jt in range(NT):
                    nc.tensor.matmul(
                        pv[:],
                        e_sb[:, jt, si : si + 128],
                        zaug[:, jt, :],
                        start=(jt == 0),
                        stop=(jt == NT - 1),
                    )
                nc.vector.tensor_copy(u_sb[:, s, :], pv[:])

        def emit_out(c):
            # normalize + residual + store for chunk c's two row-slices
            sl = slice(2 * c, 2 * c + 2)
            nc.vector.reciprocal(rec[:, sl, :], u_sb[:, sl, D : D + 1])
            nc.vector.tensor_scalar_mul(rec[:, sl, :], rec[:, sl, :], TAU)
            for s in (2 * c, 2 * c + 1):
                nc.vector.tensor_scalar_mul(
                    u_sb[:, s, 0:D], u_sb[:, s, 0:D], rec[:, s, :]
                )
                nc.vector.scalar_tensor_tensor(
                    o_sb[:, s, :],
                    zbn[:, s, :],
                    1.0 - TAU,
                    u_sb[:, s, 0:D],
                    mybir.AluOpType.mult,
                    mybir.AluOpType.add,
                )
            nc.sync.dma_start(Or[:, sl, :], o_sb[:, sl, :])

        for c in range(NCH):
            emit_sim_exp(c)
            if c > 0:
                emit_pv(c - 1)
                emit_out(c - 1)
        emit_pv(NCH - 1)
        emit_out(NCH - 1)

    return _patch_nc(nc)


# ---------------------------------------------------------------------------

_CACHE = {}


def _get_nc(c_shift: float):
    key = round(float(c_shift), 3)
    if key not in _CACHE:
        _CACHE[key] = _build_nc(key)
    return _CACHE[key]


def kernel(Z, W1, b1, W2, b2):
    Z = np.ascontiguousarray(np.asarray(Z, dtype=np.float32))
    W1 = np.ascontiguousarray(np.asarray(W1, dtype=np.float32))
    W2 = np.ascontiguousarray(np.asarray(W2, dtype=np.float32))
    b1 = np.asarray(b1, dtype=np.float32).reshape(D, 1)
    b2 = np.asarray(b2, dtype=np.float32).reshape(D, 1)

    # Host-side t (cheap) to pick the constant softmax shift C: keeps
    # exp(sim - C) inside fp32 range. sim <= max||t||^2 (Cauchy-Schwarz),
    # row maxima >= diag = ||t_i||^2.
    t = np.maximum(Z @ W1 + b1.T, 0.0) @ W2 + b2.T
    d2 = np.einsum("nd,nd->n", t, t)
    c_shift = float(min(max(d2.max() - 85.0, 0.0), d2.min() + 80.0))

    nc = _get_nc(c_shift)
    in_maps = []
    for c in range(NCORES):
        in_maps.append(
            {
                "Z": Z,
                "Zb": Z[c * BLK : (c + 1) * BLK],
                "W1": W1,
                "W2": W2,
                "b1": b1,
                "b2": b2,
            }
        )
    res = run_bass_kernel_spmd(nc, in_maps, list(range(NCORES)))
    return np.concatenate([res.results[c]["O"] for c in range(NCORES)], axis=0)
